# revision 18
# baseline (speedup 1.0000x reference)
"""AttentionBlock (GroupNorm + MHSA + proj + residual) on 8 Trainium2 cores.

Sharding: pure data-parallel over batch (B=8, one batch element per core).
Per-core dataflow (C=512, N=H*W=1024, 8 heads, hd=64, 32 groups):
  1. GroupNorm stats via bn_stats/bn_aggr (DVE) + PE group-mix matmul;
     rstd via exp(-0.5*ln(var+eps)) so ACT stays on one table set.
     Affine folded with 4*norm_w/4*norm_b -> xn (fp8e4, x4 scale).
  2. qkv weights are host-scaled by 16 and stored fp8; qk / V^T are
     fp8 DoubleRow matmuls (virtual K=256 per pass, 2x PE throughput);
     psum exits rescale by 1/64 and add biases (q pre-scaled by hd^-0.5).
  3. Scores computed TRANSPOSED in bf16: S^T[m,n] = K^T Q (row-packed
     head pairs), exp(s-2) on ACT (psum->sbuf bf16).
  4. O' = V^T-stationary matmul over P'^T (col-packed head pairs);
     softmax denominators ride as concurrent M=1 ones-matmuls into a
     shared psum bank (4-way col-tiled per quad).
  5. 64/s via ln->exp(-x+ln64) on ACT (bf16); partition-broadcast via a
     bf16 selection matmul; normalize during O' psum exit into fp8 o4
     (scale 64 keeps fp8 out of subnormals).
  6. proj as fp8 DoubleRow matmul; exit rescales by 1/1024 and fuses
     the residual add in one scalar_tensor_tensor.
"""

import math

import numpy as np
import ml_dtypes

import concourse.bass as bass
import concourse.tile as tile
from concourse import mybir
from concourse.bass_utils import run_bass_kernel_spmd
from concourse.vector_clock import ScopedClock, VectorClock

f32 = mybir.dt.float32
bf16 = mybir.dt.bfloat16
f8 = mybir.dt.float8e4
BF16 = ml_dtypes.bfloat16
FP8 = ml_dtypes.float8_e4m3  # TRN fp8_exp4: max normal 240

B, C, N = 8, 512, 1024
NH, HD, G = 8, 64, 32
EPS = 1e-5
CC = C // 128   # 4 channel chunks
OC_QK = 8       # q+k output chunks (1024 rows)
NC = 2          # n in two 512-windows
MC = 8          # m in eight 128-chunks

XS = 4.0        # xn fp8 scale
WS = 16.0       # weight fp8 scale
QV_EXIT = 1.0 / (XS * WS)          # 1/64
O4S = 16.0      # o4 fp8 scale (folded into reciprocal as ln(16) bias);
                # normalized |O| can reach ~max|V| ~ 6, so 16*6 < 240.
PROJ_EXIT = 1.0 / (WS * O4S)       # 1/256
EXP_BIAS = -2.0
_PROBE_TINY_DMA = False  # timing probe only: shrink x/out DMAs 16x
_O4_FUSE = True          # normalize via one both-psum tensor_tensor
_XN_GPSIMD = False        # xn affine+quantize on the (idle) Pool engine
_SCHRAUDOLPH_MC = (6, 7)   # mc chunks whose exp runs on DVE via the bit trick
# uint16(s*A16 + B16) == bf16 bits of e^(s-2) (Schraudolph, >>16):
_SCH_A16 = 12102203.161561485 / 65536.0
_SCH_B16 = (1064986823.0 + EXP_BIAS * 12102203.161561485) / 65536.0


def _split_multi_waits(bir_json):
    """This container's walrus build encodes at most one sync-wait command
    per TPB instruction. Engines execute in program order, so any extra
    waits can ride on NoOp instructions inserted immediately before the
    original instruction on the same engine (strictly more conservative
    ordering, semantics preserved)."""
    import orjson

    m = orjson.loads(bir_json)
    nop_id = [0]
    for fn in m.get("functions", []):
        for bb in fn.get("blocks", []):
            insts = bb.get("instructions", [])
            out = []
            for ins in insts:
                si = ins.get("sync_info") or {}
                waits = si.get("on_wait") or []
                eng = ins.get("engine", "Unassigned")
                if len(waits) > 1 and eng != "Unassigned":
                    for w in waits[:-1]:
                        nop_id[0] += 1
                        out.append(
                            {
                                "debug": ins.get("debug", 0),
                                "engine": eng,
                                "ins": [],
                                "outs": [],
                                "name": f"{ins['name']}-w{nop_id[0]}",
                                "opcode": "NoOp",
                                "sync_info": {"on_wait": [w]},
                            }
                        )
                    si = dict(si)
                    si["on_wait"] = [waits[-1]]
                    ins = dict(ins)
                    ins["sync_info"] = si
                out.append(ins)
            bb["instructions"] = out
    return orjson.dumps(m)


def _patch_tile():
    """This container's walrus accepts few sem-waits per instruction; split
    TileContext's kernel-tail drain into one drain per pending proc and
    hoist any remaining multi-waits onto NoOps at compile time."""
    if getattr(tile.TileContext, "_drain_split_patched", False):
        return

    from concourse import bass2jax, bass_utils

    orig_compile = bass_utils.compile_bir_kernel

    def compile_with_split(bir_json, tmpdir, neff_name="file.neff"):
        return orig_compile(_split_multi_waits(bir_json), tmpdir, neff_name=neff_name)

    bass_utils.compile_bir_kernel = compile_with_split
    bass2jax.compile_bir_kernel = compile_with_split

    def _drain_and_barrier_split(self, tick_clock, wait_clock):
        gc = tick_clock.global_clock
        ticks = list(gc)
        for p, t in enumerate(ticks):
            if t <= 0:
                continue
            vec = [0] * len(ticks)
            vec[p] = t
            drain_inst = self.nc.sync.drain()
            wait_clock.add_sem_waits(
                drain_inst.ins, ScopedClock({None: VectorClock(vec)})
            )
        self.nc.all_engine_barrier()
        assert self.sems is not None
        popped = self.nc._tile_sem_poison_stack.pop()
        assert popped is self._sem_poison
        self.nc.clear_and_free_semaphores(list(self.sems.allocated().values()))
        self.nc.all_engine_barrier()

    tile.TileContext._drain_and_barrier = _drain_and_barrier_split
    tile.TileContext._drain_split_patched = True


def _q8(a, scale):
    return np.clip(np.asarray(a, np.float32) * scale, -240.0, 240.0).astype(FP8)


def host_prep(x, norm_w, norm_b, qkv_w, qkv_b, proj_w, proj_b):
    """Host-side layout/dtype prep. Layout transforms + folding the
    1/sqrt(hd) attention scale into W_q/b_q (exact: 0.125 is a power of 2)
    + power-of-two fp8 scaling (exact)."""
    x = np.ascontiguousarray(np.asarray(x, np.float32)).reshape(B, C, N)
    qkv_w = np.asarray(qkv_w, np.float32)
    qkv_b = np.asarray(qkv_b, np.float32)
    scale = float(HD) ** -0.5

    wqk = qkv_w[: 2 * C].copy()
    wqk[:C] *= scale
    bqk = qkv_b[: 2 * C].copy()
    bqk[:C] *= scale
    has_bv = bool(np.any(qkv_b[2 * C :]))
    has_bp = bool(np.any(np.asarray(proj_b, np.float32)))

    common = {
        "wqkT": _q8(np.ascontiguousarray(wqk.T), WS),              # [512,1024] f8
        "wvT": _q8(np.ascontiguousarray(qkv_w[2 * C :].T), WS),    # [512,512] f8
        "wpT": _q8(np.ascontiguousarray(np.asarray(proj_w, np.float32).T), WS),
        "bqk": np.ascontiguousarray(bqk.reshape(OC_QK, 128).T).astype(np.float32),
        "bp": np.ascontiguousarray(
            np.asarray(proj_b, np.float32).reshape(CC, 128).T
        ).astype(np.float32),
        "nw": np.ascontiguousarray(
            (XS * np.asarray(norm_w, np.float32)).reshape(CC, 128).T
        ).astype(np.float32),
        "nb": np.ascontiguousarray(
            (XS * np.asarray(norm_b, np.float32)).reshape(CC, 128).T
        ).astype(np.float32),
        "gmat": _gmat(),
        "pselA": _psel(0, 32),
        "pselB": _psel(64, 96),
        "ones_col": np.ones((128, 1), BF16),
    }
    if has_bv:
        common["bv_row"] = (O4S / O4S * WS * XS * qkv_b[2 * C :]).reshape(1, C).astype(
            BF16
        )  # 64*bv: exit divides by 64
        common["ones_row"] = np.ones((1, 128), BF16)
    return common, [x[i] for i in range(B)], has_bv, has_bp


def _gmat():
    g = np.zeros((128, 128), np.float32)
    # 16 channels per group -> 8 groups per 128-chunk; inputs are
    # per-channel means so the mix weight is 1/16.
    for i in range(128):
        gi = i // 16
        g[i, gi * 16 : (gi + 1) * 16] = 1.0 / 16.0
    return g


def _psel(r0, r1):
    p = np.zeros((128, 128), BF16)
    p[r0, 0:64] = 1.0
    p[r1, 64:128] = 1.0
    return p


def build_nc(unroll=1, has_bv=False, has_bp=False):
    _patch_tile()
    DR = mybir.MatmulPerfMode.DoubleRow
    nc = bass.Bass()
    d = {}
    d["x"] = nc.declare_dram_parameter("x", [C, N], f32, isOutput=False)
    d["wqkT"] = nc.declare_dram_parameter("wqkT", [C, 2 * C], f8, isOutput=False)
    d["wvT"] = nc.declare_dram_parameter("wvT", [C, C], f8, isOutput=False)
    d["wpT"] = nc.declare_dram_parameter("wpT", [C, C], f8, isOutput=False)
    d["bqk"] = nc.declare_dram_parameter("bqk", [128, OC_QK], f32, isOutput=False)
    d["bp"] = nc.declare_dram_parameter("bp", [128, CC], f32, isOutput=False)
    d["nw"] = nc.declare_dram_parameter("nw", [128, CC], f32, isOutput=False)
    d["nb"] = nc.declare_dram_parameter("nb", [128, CC], f32, isOutput=False)
    d["gmat"] = nc.declare_dram_parameter("gmat", [128, 128], f32, isOutput=False)
    d["pselA"] = nc.declare_dram_parameter("pselA", [128, 128], bf16, isOutput=False)
    d["pselB"] = nc.declare_dram_parameter("pselB", [128, 128], bf16, isOutput=False)
    d["ones_col"] = nc.declare_dram_parameter("ones_col", [128, 1], bf16, isOutput=False)
    if has_bv:
        d["bv_row"] = nc.declare_dram_parameter("bv_row", [1, C], bf16, isOutput=False)
        d["ones_row"] = nc.declare_dram_parameter("ones_row", [1, 128], bf16, isOutput=False)
    d["out"] = nc.declare_dram_parameter("out", [C, N], f32, isOutput=True)

    with tile.TileContext(nc) as tc:
        with (
            tc.tile_pool(name="sing", bufs=1) as sing,
            tc.tile_pool(name="xp", bufs=2) as xp,
            tc.tile_pool(name="gn", bufs=4) as gnp,
            tc.tile_pool(name="pp", bufs=20) as ppp,
            tc.tile_pool(name="rqp", bufs=2) as rqp,
            tc.tile_pool(name="psA", bufs=2, space="PSUM") as psA,
            tc.tile_pool(name="psB", bufs=3, space="PSUM") as psB,
            tc.tile_pool(name="psD", bufs=1, space="PSUM") as psD,
        ):
            # ---- constants (loaded once, shared by all unrolled iters) ----
            cst = {}
            specs = [
                ("wqkT", [128, CC, 2 * C], f8),
                ("wvT", [128, CC, C], f8),
                ("wpT", [128, CC, C], f8),
                ("bqk", [128, OC_QK], f32),
                ("bp", [128, CC], f32),
                ("nw", [128, CC], f32),
                ("nb", [128, CC], f32),
                ("gmat", [128, 128], f32),
                ("pselA", [128, 128], bf16),
                ("pselB", [128, 128], bf16),
                ("ones_col", [128, 1], bf16),
            ]
            if has_bv:
                specs += [("bv_row", [1, C], bf16), ("ones_row", [1, 128], bf16)]
            for name, shape, dt in specs:
                t = sing.tile(shape, dt, tag=name, name=name)
                src = d[name]
                if len(shape) == 3:
                    nc.sync.dma_start(
                        out=t, in_=src.rearrange("(cc p) o -> p cc o", p=128)
                    )
                else:
                    nc.sync.dma_start(out=t, in_=src[:])
                cst[name] = t
            for cname, val in (
                ("eps", EPS),
                ("neg2", EXP_BIAS),
                ("ln64", math.log(O4S)),
                ("iqv", QV_EXIT),
                ("ipj", PROJ_EXIT),
                ("schA", _SCH_A16),
                ("schB", _SCH_B16),
            ):
                t = sing.tile([128, 1], f32, tag=cname, name=cname)
                nc.vector.memset(t, val)
                cst[cname] = t

            s_ps = psD.tile([128, 512], f32, tag="s", name="s_ps")
            rb_ps = s_ps  # rb broadcasts reuse the sums bank post-reciprocal
            # stale psum rows must stay finite through ln/exp/psel-mm.
            nc.vector.memset(s_ps, 1.0)

            for _ in range(unroll):
                _body(nc, tc, d, cst, sing, xp, gnp, ppp, rqp, psA, psB,
                      s_ps, rb_ps, has_bv, has_bp)
    return nc


def _body(nc, tc, d, cst, sing, xp, gnp, ppp, rqp, psA, psB, s_ps, rb_ps,
          has_bv, has_bp):
    AF = mybir.ActivationFunctionType
    OP = mybir.AluOpType
    DR = mybir.MatmulPerfMode.DoubleRow

    x4 = []
    for cc in range(CC):
        xt = xp.tile([128, N], f32, tag=f"x{cc}", name=f"x{cc}")
        if _PROBE_TINY_DMA:
            nc.gpsimd.dma_start(
                out=xt[:, 0:64], in_=d["x"][cc * 128 : (cc + 1) * 128, 0:64]
            )
        else:
            nc.gpsimd.dma_start(out=xt, in_=d["x"][cc * 128 : (cc + 1) * 128, :])
        x4.append(xt)

    # ---------------- GroupNorm -> xn (fp8, x4 scale) ----------------
    xn = xp.tile([128, CC, N], f8, tag="xn", name="xn")
    for cc in range(CC):
        bst = gnp.tile([128, 2, 6], f32, tag="bst", name="bst")
        nc.vector.bn_stats(out=bst[:, 0, :], in_=x4[cc][:, 0:512])
        nc.vector.bn_stats(out=bst[:, 1, :], in_=x4[cc][:, 512:1024])
        mv = gnp.tile([128, 2], f32, tag="mv", name="mv")
        nc.vector.bn_aggr(out=mv, in_=bst)
        # ms = [mean, E[x^2]] per channel
        ms = gnp.tile([128, 2], f32, tag="ms", name="ms")
        nc.vector.tensor_copy(out=ms[:, 0:1], in_=mv[:, 0:1])
        nc.vector.scalar_tensor_tensor(
            out=ms[:, 1:2],
            in0=mv[:, 0:1],
            scalar=mv[:, 0:1],
            in1=mv[:, 1:2],
            op0=OP.mult,
            op1=OP.add,
        )
        gst_ps = psB.tile([128, 2], f32, tag="bank", name="gst_ps")
        nc.tensor.matmul(gst_ps, cst["gmat"], ms, start=True, stop=True)
        gst = gnp.tile([128, 2], f32, tag="gst", name="gst")
        nc.vector.tensor_copy(out=gst, in_=gst_ps)
        # negvar = mean_g^2 - E_g[x^2];  rstd = exp(-0.5*ln(var+eps))
        negvar = gnp.tile([128, 1], f32, tag="negvar", name="negvar")
        nc.vector.scalar_tensor_tensor(
            out=negvar,
            in0=gst[:, 0:1],
            scalar=gst[:, 0:1],
            in1=gst[:, 1:2],
            op0=OP.mult,
            op1=OP.subtract,
        )
        rstd = gnp.tile([128, 1], f32, tag="rstd", name="rstd")
        nc.scalar.activation(out=rstd, in_=negvar, func=AF.Ln, bias=cst["eps"], scale=-1.0)
        nc.scalar.activation(out=rstd, in_=rstd, func=AF.Exp, scale=-0.5)
        aa = gnp.tile([128, 1], f32, tag="aa", name="aa")
        nc.vector.tensor_mul(out=aa, in0=rstd, in1=cst["nw"][:, cc : cc + 1])
        # bbn = mean_g*A - 4*norm_b   (applied as x*A - bbn)
        bbn = gnp.tile([128, 1], f32, tag="bbn", name="bbn")
        nc.vector.scalar_tensor_tensor(
            out=bbn,
            in0=gst[:, 0:1],
            scalar=aa,
            in1=cst["nb"][:, cc : cc + 1],
            op0=OP.mult,
            op1=OP.subtract,
        )
        eng = nc.gpsimd if _XN_GPSIMD else nc.vector
        eng.tensor_scalar(
            out=xn[:, cc, :], in0=x4[cc], scalar1=aa, scalar2=bbn,
            op0=OP.mult, op1=OP.subtract,
        )

    # ---------------- q, k generation (fp8 DoubleRow) ----------------
    qk = []
    for oc in range(OC_QK):
        qkt = xp.tile([128, N], bf16, tag=f"qk{oc}", name=f"qk{oc}")
        qk.append(qkt)
        for nci in range(NC):
            nwin = slice(nci * 512, (nci + 1) * 512)
            ps = psB.tile([128, 512], f32, tag="bank", name="bank")
            for ci in (0, 2):
                nc.tensor.matmul(
                    ps,
                    cst["wqkT"][:, ci : ci + 2, oc * 128 : (oc + 1) * 128],
                    xn[:, ci : ci + 2, nwin],
                    start=(ci == 0),
                    stop=(ci == 2),
                    perf_mode=DR,
                )
            nc.vector.tensor_scalar(
                out=qkt[:, nwin], in0=ps, scalar1=cst["iqv"],
                scalar2=cst["bqk"][:, oc : oc + 1], op0=OP.mult, op1=OP.add,
            )

    # ---------------- V^T: V^T[n, vo] = xn^T @ wv^T (fp8 DoubleRow) ----------------
    vT = []
    for mci in range(MC):
        vt = xp.tile([128, C], bf16, tag=f"vT{mci}", name=f"vT{mci}")
        vT.append(vt)
        ps = psB.tile([128, 512], f32, tag="bank", name="bank")
        for ci in (0, 2):
            nc.tensor.matmul(
                ps,
                xn[:, ci : ci + 2, mci * 128 : (mci + 1) * 128],
                cst["wvT"][:, ci : ci + 2, :],
                start=(ci == 0),
                stop=(ci == 2) and not has_bv,
                perf_mode=DR,
            )
        if has_bv:
            nc.tensor.matmul(ps, cst["ones_row"], cst["bv_row"], start=False, stop=True)
        nc.vector.tensor_scalar(
            out=vt, in0=ps, scalar1=cst["iqv"], scalar2=None, op0=OP.mult,
        )

    # ---------------- attention ----------------
    o4 = xp.tile([128, CC, N], f8, tag="o4", name="o4")

    for nci in range(NC):
        nwin = slice(nci * 512, (nci + 1) * 512)
        for q in range(2):  # quad of heads 4q..4q+3
            pp_tiles = {}
            for pi in range(2):  # pair within quad
                h0 = 4 * q + 2 * pi       # even head -> partitions 0:64
                h1 = h0 + 1               # odd head  -> partitions 64:128
                for mc in range(MC):
                    sg = psA.tile([128, 1024], f32, tag="sg", name="sg")
                    for sl, h in enumerate((h0, h1)):
                        hp = (h % 2) * 64
                        nc.tensor.matmul(
                            sg[:, sl * 512 : (sl + 1) * 512],
                            qk[4 + h // 2][hp : hp + 64, mc * 128 : (mc + 1) * 128],
                            qk[h // 2][hp : hp + 64, nwin],
                            start=True,
                            stop=True,
                            tile_position=(hp, 0),
                        )
                    pt = ppp.tile([128, 1024], bf16, tag="pp", name="pp")
                    if mc in _SCHRAUDOLPH_MC:
                        # exp on DVE: uint16(s*A+B) bits == bf16(e^(s-2))
                        nc.vector.tensor_scalar(
                            out=pt.bitcast(mybir.dt.uint16), in0=sg,
                            scalar1=cst["schA"], scalar2=cst["schB"],
                            op0=OP.mult, op1=OP.add,
                        )
                    else:
                        nc.scalar.activation(out=pt, in_=sg, func=AF.Exp, bias=cst["neg2"])
                    pp_tiles[(pi, mc)] = pt

            att = [psB.tile([128, 512], f32, tag="bank", name="bank") for _ in range(2)]
            for mc in range(MC):
                for pi in range(2):
                    h0 = 4 * q + 2 * pi
                    pt = pp_tiles[(pi, mc)]
                    for hh in range(2):  # head within pair
                        nc.tensor.matmul(
                            att[pi][hh * 64 : (hh + 1) * 64, :],
                            vT[mc][:, (h0 + hh) * 64 : (h0 + hh + 1) * 64],
                            pt[:, hh * 512 : (hh + 1) * 512],
                            start=(mc == 0),
                            stop=(mc == MC - 1),
                            tile_position=(0, hh * 64),
                            skip_group_check=True,
                        )
            # denominators after: 4-way col-tiled M=1 ones matmuls, so the
            # attnV stream is not blocked by the s_ps recip/rbc chain
            for mc in range(MC):
                for pi in range(2):
                    pt = pp_tiles[(pi, mc)]
                    for hh in range(2):
                        j = 2 * pi + hh
                        nc.tensor.matmul(
                            s_ps[32 * j : 32 * j + 1, :],
                            cst["ones_col"],
                            pt[:, hh * 512 : (hh + 1) * 512],
                            start=(mc == 0),
                            stop=(mc == MC - 1),
                            tile_position=(0, 32 * j),
                            skip_group_check=True,
                        )

            # r = 16/s via exp(-ln(s)+ln16): both in natural_log_exp set
            rq = rqp.tile([128, 512], bf16, tag="rq", name="rq")
            scr = rqp.tile([128, 512], f32, tag="scr", name="scr")
            nc.scalar.activation(out=scr, in_=s_ps, func=AF.Ln)
            nc.scalar.activation(out=rq, in_=scr, func=AF.Exp, scale=-1.0, bias=cst["ln64"])

            for pi in range(2):
                if _O4_FUSE:
                    # psel matmul broadcasts r rows; ACT (cheaper op, engine
                    # less loaded than DVE) stages rb to SBUF so the
                    # normalize is one single-psum-operand DVE op that also
                    # exits O' straight into fp8.
                    psel = cst["pselA"] if pi == 0 else cst["pselB"]
                    nc.tensor.matmul(rb_ps, psel, rq, start=True, stop=True)
                    rb_sb = rqp.tile([128, 512], bf16, tag="rb", name="rb_sb")
                    nc.scalar.copy(out=rb_sb, in_=rb_ps)
                    nc.vector.tensor_tensor(
                        out=o4[:, 2 * q + pi, nwin],
                        in0=att[pi],
                        in1=rb_sb,
                        op=OP.mult,
                    )
                else:
                    psel = cst["pselA"] if pi == 0 else cst["pselB"]
                    nc.tensor.matmul(rb_ps, psel, rq, start=True, stop=True)
                    oscr = rqp.tile([128, 512], bf16, tag="oscr", name="oscr", bufs=3)
                    nc.vector.tensor_copy(out=oscr, in_=att[pi])
                    nc.vector.tensor_tensor(
                        out=o4[:, 2 * q + pi, nwin],
                        in0=oscr,
                        in1=rb_ps,
                        op=OP.mult,
                    )

    # ---------------- proj (fp8 DoubleRow) + residual ----------------
    for nci in range(NC):
        nwin = slice(nci * 512, (nci + 1) * 512)
        for oc in range(CC):
            ps = psB.tile([128, 512], f32, tag="bank", name="bank")
            for ci in (0, 2):
                nc.tensor.matmul(
                    ps,
                    cst["wpT"][:, ci : ci + 2, oc * 128 : (oc + 1) * 128],
                    o4[:, ci : ci + 2, nwin],
                    start=(ci == 0),
                    stop=(ci == 2),
                    perf_mode=DR,
                )
            ob = gnp.tile([128, 512], f32, tag="ob", name="ob", bufs=4)
            if has_bp:
                ob2 = gnp.tile([128, 512], f32, tag="ob2", name="ob2", bufs=2)
                nc.vector.tensor_scalar(
                    out=ob2, in0=ps, scalar1=cst["ipj"],
                    scalar2=cst["bp"][:, oc : oc + 1], op0=OP.mult, op1=OP.add,
                )
                nc.vector.tensor_tensor(
                    out=ob, in0=ob2, in1=x4[oc][:, nwin], op=OP.add
                )
            else:
                nc.vector.scalar_tensor_tensor(
                    out=ob,
                    in0=ps,
                    scalar=cst["ipj"],
                    in1=x4[oc][:, nwin],
                    op0=OP.mult,
                    op1=OP.add,
                )
            if _PROBE_TINY_DMA:
                nc.sync.dma_start(
                    out=d["out"][oc * 128 : (oc + 1) * 128, nci * 512 : nci * 512 + 64],
                    in_=ob[:, 0:64],
                )
            else:
                nc.sync.dma_start(out=d["out"][oc * 128 : (oc + 1) * 128, nwin], in_=ob)


_BUILT = None
_BUILT_KEY = None


def kernel(**inputs):
    global _BUILT, _BUILT_KEY
    common, xs, has_bv, has_bp = host_prep(**inputs)
    key = (has_bv, has_bp)
    if _BUILT is None or _BUILT_KEY != key:
        _BUILT = build_nc(unroll=1, has_bv=has_bv, has_bp=has_bp)
        _BUILT_KEY = key
    nc = _BUILT
    in_maps = [dict(common, x=xs[i]) for i in range(B)]
    res = run_bass_kernel_spmd(nc, in_maps, core_ids=list(range(B)))
    out = np.stack([res.results[i]["out"] for i in range(B)], axis=0)
    return out.reshape(B, C, 32, 32).astype(np.float32)
